# revision 8
# baseline (speedup 1.0000x reference)
"""AGA flash-attention (routed slot attention) TRN2 kernel.

Data-parallel over tokens: 16384 tokens split across 8 NeuronCores, slot
memory (keys/values/reliability) replicated. Per core, per 128-token tile:
  1. PE: router scores r = qT.T @ keysT accumulated with a K=1 outer-product
     bias matmul (bias = ln(reliability + eps) per slot).
  2. ACT: drain PSUM -> SBUF.
  3. DVE: max8 (top-8 values, descending) + find_index8 (their slot indices).
  4. GPSIMD indirect DMA: gather bias at the 8 indices (per-token) and the
     8 value rows (bf16) per token.
  5. ACT: e8 = exp((r8-b8)*SCALE) with accumulated denominator; DVE recip.
  6. PE: out = sum_k diag(e8_k) @ V_rows_k (psum f32), drained with a
     per-token 1/denom scale on ACT.  attn_weights = e8 * (1/denom).
Output is packed [tokens, 1024+8] (values || weights) and split on host.
"""

import sys

for _p in ("/opt/trn_rl_repo",):
    if _p not in sys.path:
        sys.path.append(_p)

import numpy as np
import ml_dtypes

import concourse.bass as bass
import concourse.bacc as bacc
import concourse.mybir as mybir
from concourse.tile import TileContext
from concourse.bass_utils import run_bass_kernel_spmd
from concourse import masks

F32 = mybir.dt.float32
BF16 = mybir.dt.bfloat16
U32 = mybir.dt.uint32
AF = mybir.ActivationFunctionType
ALU = mybir.AluOpType

B, S, D, N, H, K = 4, 4096, 128, 4096, 1024, 8
N_CORES = 8
TOKENS = B * S
TPC = TOKENS // N_CORES  # tokens per core
P = 128
SCALE = 1.0 / float(np.sqrt(D))
EPS = 1e-10
OUTW = H + K
W = 1028  # augmented row: 1024 values + bias + pad


def build(tpc=TPC):
    n_tiles = tpc // P
    nc = bacc.Bacc("TRN2", target_bir_lowering=False, debug=False)
    qT = nc.dram_tensor("qT", [D, tpc], F32, kind="ExternalInput")
    keysT = nc.dram_tensor("keysT", [D, N], F32, kind="ExternalInput")
    vals = nc.dram_tensor("vals", [N, W], BF16, kind="ExternalInput")
    rel = nc.dram_tensor("rel", [1, N], F32, kind="ExternalInput")
    out = nc.dram_tensor("out", [tpc, OUTW], F32, kind="ExternalOutput")
    bias_d = nc.dram_tensor("bias_in", [1, N], F32, kind="ExternalInput")

    with TileContext(nc) as tc:
        with (
            tc.tile_pool(name="const", bufs=1) as cpool,
            tc.tile_pool(name="scores", bufs=2) as spool,
            tc.tile_pool(name="gather", bufs=2) as gpool,
            tc.tile_pool(name="outp", bufs=2) as opool,
            tc.tile_pool(name="small", bufs=3) as smpool,
            tc.tile_pool(name="diag", bufs=3) as dpool,
            tc.tile_pool(name="ps_s", bufs=2, space="PSUM") as pspool,
            tc.tile_pool(name="ps_o", bufs=2, space="PSUM") as popool,
        ):
            qT_sb = cpool.tile([D, tpc], F32)
            nc.sync.dma_start(out=qT_sb[:], in_=qT.ap())
            keysT_sb = cpool.tile([D, N], F32)
            nc.sync.dma_start(out=keysT_sb[:], in_=keysT.ap())
            rel_sb = cpool.tile([1, N], F32)
            nc.sync.dma_start(out=rel_sb[:], in_=rel.ap())
            bias_sb = cpool.tile([1, N], F32)
            nc.sync.dma_start(out=bias_sb[:], in_=bias_d.ap())
            ones_sb = cpool.tile([1, P], F32)
            nc.vector.memset(ones_sb[:], 1.0)
            ident = cpool.tile([P, P], BF16)
            masks.make_identity(nc, ident[:])

            for i in range(n_tiles):
                ssb = spool.tile([P, N], F32)
                for c in range(N // 1024):
                    ps = pspool.tile([P, 1024], F32)
                    for h2 in range(2):
                        sl = slice(c * 1024 + h2 * 512, c * 1024 + (h2 + 1) * 512)
                        psl = ps[:, h2 * 512 : (h2 + 1) * 512]
                        nc.tensor.matmul(
                            out=psl,
                            lhsT=qT_sb[:, i * P : (i + 1) * P],
                            rhs=keysT_sb[:, sl],
                            start=True,
                            stop=False,
                        )
                        nc.tensor.matmul(
                            out=psl,
                            lhsT=ones_sb[:],
                            rhs=bias_sb[:, sl],
                            start=False,
                            stop=True,
                        )
                    nc.scalar.activation(
                        out=ssb[:, c * 1024 : (c + 1) * 1024], in_=ps[:], func=AF.Copy
                    )

                r8 = smpool.tile([P, K], F32, tag="r8")
                idx = smpool.tile([P, K], U32, tag="idx")
                nc.vector.max(out=r8[:], in_=ssb[:])
                nc.vector.max_index(out=idx[:], in_max=r8[:], in_values=ssb[:])

                g = gpool.tile([P, K * W], BF16)
                for k in range(K):
                    nc.gpsimd.indirect_dma_start(
                        out=g[:, k * W : (k + 1) * W],
                        out_offset=None,
                        in_=vals.ap(),
                        in_offset=bass.IndirectOffsetOnAxis(
                            ap=idx[:, k : k + 1], axis=0
                        ),
                    )
                g3 = g[:].rearrange("p (k w) -> p k w", w=W)
                d8 = smpool.tile([P, K], F32, tag="d8")
                nc.vector.tensor_sub(d8[:], r8[:], g3[:, :, H : H + 1])
                e8 = smpool.tile([P, K], F32, tag="e8")
                den = smpool.tile([P, 1], F32, tag="den")
                nc.scalar.activation(
                    out=e8[:], in_=d8[:], func=AF.Exp, scale=SCALE, accum_out=den[:]
                )
                winv = smpool.tile([P, 1], F32, tag="winv")
                nc.vector.reciprocal(out=winv[:], in_=den[:])

                osb = opool.tile([P, OUTW], F32)
                nc.vector.tensor_scalar(
                    out=osb[:, H:], in0=e8[:], scalar1=winv[:], scalar2=None,
                    op0=ALU.mult,
                )

                po = popool.tile([P, H], F32)
                for k in range(K):
                    dg = dpool.tile([P, P], BF16, tag="dg")
                    nc.gpsimd.tensor_scalar(
                        out=dg[:], in0=ident[:], scalar1=e8[:, k : k + 1],
                        scalar2=None, op0=ALU.mult,
                    )
                    for h2 in range(2):
                        nc.tensor.matmul(
                            out=po[:, h2 * 512 : (h2 + 1) * 512],
                            lhsT=dg[:],
                            rhs=g[:, k * W + h2 * 512 : k * W + (h2 + 1) * 512],
                            start=(k == 0),
                            stop=(k == K - 1),
                        )
                nc.scalar.activation(
                    out=osb[:, :H], in_=po[:], func=AF.Copy, scale=winv[:]
                )
                nc.sync.dma_start(
                    out=out.ap()[i * P : (i + 1) * P, :], in_=osb[:]
                )
    nc.compile()
    return nc


def make_in_maps(query, keys, values, reliability, tpc=TPC, n_cores=N_CORES):
    query = np.asarray(query, dtype=np.float32)
    keys = np.asarray(keys, dtype=np.float32)
    values = np.asarray(values, dtype=np.float32)
    reliability = np.asarray(reliability, dtype=np.float32)
    qf = query.reshape(-1, D)
    keysT = np.ascontiguousarray(keys.T)
    bias_f = np.log(reliability.reshape(N) + EPS).astype(np.float32)
    vals16 = np.zeros((N, W), dtype=ml_dtypes.bfloat16)
    vals16[:, :H] = values.astype(ml_dtypes.bfloat16)
    vals16[:, H] = bias_f.astype(ml_dtypes.bfloat16)
    bias_row = np.ascontiguousarray(bias_f.reshape(1, N))
    rel2 = reliability.reshape(1, N)
    in_maps = []
    for c in range(n_cores):
        shard = qf[c * tpc : (c + 1) * tpc]
        in_maps.append(
            {
                "qT": np.ascontiguousarray(shard.T),
                "keysT": keysT,
                "vals": vals16,
                "rel": rel2,
                "bias_in": bias_row,
            }
        )
    return in_maps


_CACHED_NC = None


def _get_nc():
    global _CACHED_NC
    if _CACHED_NC is None:
        _CACHED_NC = build()
    return _CACHED_NC


def run(query, keys, values, reliability, trace=False, **run_kwargs):
    nc = _get_nc()
    in_maps = make_in_maps(query, keys, values, reliability)
    res = run_bass_kernel_spmd(
        nc, in_maps, core_ids=list(range(N_CORES)), trace=trace, **run_kwargs
    )
    full = np.concatenate([res.results[c]["out"] for c in range(N_CORES)], axis=0)
    output = np.ascontiguousarray(full[:, :H]).reshape(B, S, H)
    attn = np.ascontiguousarray(full[:, H:]).reshape(B, S, K)
    return (output, attn), res


def kernel(query, keys, values, reliability):
    (output, attn), _ = run(query, keys, values, reliability, trace=False)
    return output, attn


# revision 9
# speedup vs baseline: 1.4082x; 1.4082x over previous
"""AGA flash-attention (routed slot attention) TRN2 kernel.

Data-parallel over tokens: 16384 tokens split across 8 NeuronCores, slot
memory (keys/values/reliability) replicated. Per core, per 128-token tile:
  1. PE: router scores r = qT.T @ keysT accumulated with a K=1 outer-product
     bias matmul (bias = ln(reliability + eps) per slot).
  2. ACT: drain PSUM -> SBUF.
  3. DVE: max8 (top-8 values, descending) + find_index8 (their slot indices).
  4. GPSIMD indirect DMA: gather bias at the 8 indices (per-token) and the
     8 value rows (bf16) per token.
  5. ACT: e8 = exp((r8-b8)*SCALE) with accumulated denominator; DVE recip.
  6. PE: out = sum_k diag(e8_k) @ V_rows_k (psum f32), drained with a
     per-token 1/denom scale on ACT.  attn_weights = e8 * (1/denom).
Output is packed [tokens, 1024+8] (values || weights) and split on host.
"""

import sys

for _p in ("/opt/trn_rl_repo",):
    if _p not in sys.path:
        sys.path.append(_p)

import numpy as np
import ml_dtypes

import concourse.bass as bass
import concourse.bacc as bacc
import concourse.mybir as mybir
from concourse.tile import TileContext
from concourse.bass_utils import run_bass_kernel_spmd
from concourse import masks

F32 = mybir.dt.float32
BF16 = mybir.dt.bfloat16
U32 = mybir.dt.uint32
AF = mybir.ActivationFunctionType
ALU = mybir.AluOpType

B, S, D, N, H, K = 4, 4096, 128, 4096, 1024, 8
N_CORES = 8
TOKENS = B * S
TPC = TOKENS // N_CORES  # tokens per core
P = 128
SCALE = 1.0 / float(np.sqrt(D))
EPS = 1e-10
OUTW = H + K
W = 1028  # augmented row: 1024 values + bias + pad


def build(tpc=TPC):
    n_tiles = tpc // P
    nc = bacc.Bacc("TRN2", target_bir_lowering=False, debug=False)
    qT = nc.dram_tensor("qT", [D, tpc], F32, kind="ExternalInput")
    keysT = nc.dram_tensor("keysT", [D, N], F32, kind="ExternalInput")
    vals = nc.dram_tensor("vals", [N, W], BF16, kind="ExternalInput")
    rel = nc.dram_tensor("rel", [1, N], F32, kind="ExternalInput")
    out = nc.dram_tensor("out", [tpc, OUTW], F32, kind="ExternalOutput")
    bias_d = nc.dram_tensor("bias_in", [1, N], F32, kind="ExternalInput")

    with TileContext(nc) as tc:
        with (
            tc.tile_pool(name="const", bufs=1) as cpool,
            tc.tile_pool(name="scores", bufs=2) as spool,
            tc.tile_pool(name="gather", bufs=2) as gpool,
            tc.tile_pool(name="outp", bufs=2) as opool,
            tc.tile_pool(name="small", bufs=3) as smpool,
            tc.tile_pool(name="diag", bufs=3) as dpool,
            tc.tile_pool(name="ps_s", bufs=2, space="PSUM") as pspool,
            tc.tile_pool(name="ps_o", bufs=2, space="PSUM") as popool,
        ):
            qT_sb = cpool.tile([D, tpc], F32)
            nc.sync.dma_start(out=qT_sb[:], in_=qT.ap())
            keysT_sb = cpool.tile([D, N], F32)
            nc.sync.dma_start(out=keysT_sb[:], in_=keysT.ap())
            rel_sb = cpool.tile([1, N], F32)
            nc.sync.dma_start(out=rel_sb[:], in_=rel.ap())
            bias_sb = cpool.tile([1, N], F32)
            nc.sync.dma_start(out=bias_sb[:], in_=bias_d.ap())
            ones_sb = cpool.tile([1, P], F32)
            nc.vector.memset(ones_sb[:], 1.0)
            ident = cpool.tile([P, P], BF16)
            masks.make_identity(nc, ident[:])

            for i in range(n_tiles):
                ssb = spool.tile([P, N], F32)
                for c in range(N // 1024):
                    ps = pspool.tile([P, 1024], F32)
                    for h2 in range(2):
                        sl = slice(c * 1024 + h2 * 512, c * 1024 + (h2 + 1) * 512)
                        psl = ps[:, h2 * 512 : (h2 + 1) * 512]
                        nc.tensor.matmul(
                            out=psl,
                            lhsT=qT_sb[:, i * P : (i + 1) * P],
                            rhs=keysT_sb[:, sl],
                            start=True,
                            stop=False,
                        )
                        nc.tensor.matmul(
                            out=psl,
                            lhsT=ones_sb[:],
                            rhs=bias_sb[:, sl],
                            start=False,
                            stop=True,
                        )
                    nc.scalar.activation(
                        out=ssb[:, c * 1024 : (c + 1) * 1024], in_=ps[:], func=AF.Copy
                    )

                r8 = smpool.tile([P, K], F32, tag="r8")
                idx = smpool.tile([P, K], U32, tag="idx")
                nc.vector.max(out=r8[:], in_=ssb[:])
                nc.vector.max_index(out=idx[:], in_max=r8[:], in_values=ssb[:])

                g = gpool.tile([P, K * W], BF16)
                for k in range(K):
                    nc.gpsimd.indirect_dma_start(
                        out=g[:, k * W : (k + 1) * W],
                        out_offset=None,
                        in_=vals.ap(),
                        in_offset=bass.IndirectOffsetOnAxis(
                            ap=idx[:, k : k + 1], axis=0
                        ),
                    )
                g3 = g[:].rearrange("p (k w) -> p k w", w=W)
                d8 = smpool.tile([P, K], F32, tag="d8")
                nc.vector.tensor_sub(d8[:], r8[:], g3[:, :, H : H + 1])
                e8 = smpool.tile([P, K], F32, tag="e8")
                den = smpool.tile([P, 1], F32, tag="den")
                nc.scalar.activation(
                    out=e8[:], in_=d8[:], func=AF.Exp, scale=SCALE, accum_out=den[:]
                )
                winv = smpool.tile([P, 1], F32, tag="winv")
                nc.vector.reciprocal(out=winv[:], in_=den[:])

                osb = opool.tile([P, OUTW], F32)
                nc.vector.tensor_scalar(
                    out=osb[:, H:], in0=e8[:], scalar1=winv[:], scalar2=None,
                    op0=ALU.mult,
                )

                po = popool.tile([P, H], F32)
                for k in range(K):
                    dg = dpool.tile([P, P], BF16, tag="dg")
                    nc.scalar.activation(
                        out=dg[:], in_=ident[:], func=AF.Copy,
                        scale=e8[:, k : k + 1],
                    )
                    for h2 in range(2):
                        nc.tensor.matmul(
                            out=po[:, h2 * 512 : (h2 + 1) * 512],
                            lhsT=dg[:],
                            rhs=g[:, k * W + h2 * 512 : k * W + (h2 + 1) * 512],
                            start=(k == 0),
                            stop=(k == K - 1),
                        )
                nc.scalar.activation(
                    out=osb[:, :H], in_=po[:], func=AF.Copy, scale=winv[:]
                )
                nc.sync.dma_start(
                    out=out.ap()[i * P : (i + 1) * P, :], in_=osb[:]
                )
    nc.compile()
    return nc


def make_in_maps(query, keys, values, reliability, tpc=TPC, n_cores=N_CORES):
    query = np.asarray(query, dtype=np.float32)
    keys = np.asarray(keys, dtype=np.float32)
    values = np.asarray(values, dtype=np.float32)
    reliability = np.asarray(reliability, dtype=np.float32)
    qf = query.reshape(-1, D)
    keysT = np.ascontiguousarray(keys.T)
    bias_f = np.log(reliability.reshape(N) + EPS).astype(np.float32)
    vals16 = np.zeros((N, W), dtype=ml_dtypes.bfloat16)
    vals16[:, :H] = values.astype(ml_dtypes.bfloat16)
    vals16[:, H] = bias_f.astype(ml_dtypes.bfloat16)
    bias_row = np.ascontiguousarray(bias_f.reshape(1, N))
    rel2 = reliability.reshape(1, N)
    in_maps = []
    for c in range(n_cores):
        shard = qf[c * tpc : (c + 1) * tpc]
        in_maps.append(
            {
                "qT": np.ascontiguousarray(shard.T),
                "keysT": keysT,
                "vals": vals16,
                "rel": rel2,
                "bias_in": bias_row,
            }
        )
    return in_maps


_CACHED_NC = None


def _get_nc():
    global _CACHED_NC
    if _CACHED_NC is None:
        _CACHED_NC = build()
    return _CACHED_NC


def run(query, keys, values, reliability, trace=False, **run_kwargs):
    nc = _get_nc()
    in_maps = make_in_maps(query, keys, values, reliability)
    res = run_bass_kernel_spmd(
        nc, in_maps, core_ids=list(range(N_CORES)), trace=trace, **run_kwargs
    )
    full = np.concatenate([res.results[c]["out"] for c in range(N_CORES)], axis=0)
    output = np.ascontiguousarray(full[:, :H]).reshape(B, S, H)
    attn = np.ascontiguousarray(full[:, H:]).reshape(B, S, K)
    return (output, attn), res


def kernel(query, keys, values, reliability):
    (output, attn), _ = run(query, keys, values, reliability, trace=False)
    return output, attn


# revision 10
# speedup vs baseline: 2.3145x; 1.6435x over previous
"""AGA flash-attention (routed slot attention) TRN2 kernel.

Data-parallel over tokens: 16384 tokens split across 8 NeuronCores, slot
memory (keys/values/reliability) replicated. Per core, per 128-token tile:
  1. PE: router scores r = qT.T @ keysT accumulated with a K=1 outer-product
     bias matmul (bias = ln(reliability + eps) per slot).
  2. ACT: drain PSUM -> SBUF.
  3. DVE: max8 (top-8 values, descending) + find_index8 (their slot indices).
  4. GPSIMD indirect DMA: gather bias at the 8 indices (per-token) and the
     8 value rows (bf16) per token.
  5. ACT: e8 = exp((r8-b8)*SCALE) with accumulated denominator; DVE recip.
  6. PE: out = sum_k diag(e8_k) @ V_rows_k (psum f32), drained with a
     per-token 1/denom scale on ACT.  attn_weights = e8 * (1/denom).
Output is packed [tokens, 1024+8] (values || weights) and split on host.
"""

import sys

for _p in ("/opt/trn_rl_repo",):
    if _p not in sys.path:
        sys.path.append(_p)

import numpy as np
import ml_dtypes

import concourse.bass as bass
import concourse.bacc as bacc
import concourse.mybir as mybir
from concourse.tile import TileContext
from concourse.bass_utils import run_bass_kernel_spmd
from concourse import masks

F32 = mybir.dt.float32
BF16 = mybir.dt.bfloat16
U32 = mybir.dt.uint32
AF = mybir.ActivationFunctionType
ALU = mybir.AluOpType

B, S, D, N, H, K = 4, 4096, 128, 4096, 1024, 8
N_CORES = 8
TOKENS = B * S
TPC = TOKENS // N_CORES  # tokens per core
P = 128
SCALE = 1.0 / float(np.sqrt(D))
EPS = 1e-10
OUTW = H + K
W = 1028  # augmented row: 1024 values + bias + pad


def build(tpc=TPC):
    n_tiles = tpc // P
    nc = bacc.Bacc("TRN2", target_bir_lowering=False, debug=False)
    qT = nc.dram_tensor("qT", [D, tpc], F32, kind="ExternalInput")
    keysT = nc.dram_tensor("keysT", [D, N], F32, kind="ExternalInput")
    vals = nc.dram_tensor("vals", [N, W], BF16, kind="ExternalInput")
    rel = nc.dram_tensor("rel", [1, N], F32, kind="ExternalInput")
    out = nc.dram_tensor("out", [tpc, OUTW], F32, kind="ExternalOutput")
    bias_d = nc.dram_tensor("bias_in", [1, N], F32, kind="ExternalInput")

    with TileContext(nc) as tc:
        with (
            tc.tile_pool(name="const", bufs=1) as cpool,
            tc.tile_pool(name="scores", bufs=2) as spool,
            tc.tile_pool(name="gather", bufs=2) as gpool,
            tc.tile_pool(name="outp", bufs=2) as opool,
            tc.tile_pool(name="small", bufs=3) as smpool,
            tc.tile_pool(name="diag", bufs=3) as dpool,
            tc.tile_pool(name="ps_s", bufs=2, space="PSUM") as pspool,
            tc.tile_pool(name="ps_o", bufs=2, space="PSUM") as popool,
        ):
            qT_sb = cpool.tile([D, tpc], F32)
            nc.sync.dma_start(out=qT_sb[:], in_=qT.ap())
            keysT_sb = cpool.tile([D, N], F32)
            nc.sync.dma_start(out=keysT_sb[:], in_=keysT.ap())
            rel_sb = cpool.tile([1, N], F32)
            nc.sync.dma_start(out=rel_sb[:], in_=rel.ap())
            bias_sb = cpool.tile([1, N], F32)
            nc.sync.dma_start(out=bias_sb[:], in_=bias_d.ap())
            ones_sb = cpool.tile([1, P], F32)
            nc.vector.memset(ones_sb[:], 1.0)
            ident = cpool.tile([P, P], BF16)
            masks.make_identity(nc, ident[:])

            for i in range(n_tiles):
                ssb = spool.tile([P, N], F32)
                for c in range(N // 1024):
                    ps = pspool.tile([P, 1024], F32)
                    # group matmuls by stationary operand to avoid LDWEIGHTS
                    # ping-pong: both qT matmuls, then both bias matmuls
                    for h2 in range(2):
                        sl = slice(c * 1024 + h2 * 512, c * 1024 + (h2 + 1) * 512)
                        nc.tensor.matmul(
                            out=ps[:, h2 * 512 : (h2 + 1) * 512],
                            lhsT=qT_sb[:, i * P : (i + 1) * P],
                            rhs=keysT_sb[:, sl],
                            start=True,
                            stop=False,
                        )
                    for h2 in range(2):
                        sl = slice(c * 1024 + h2 * 512, c * 1024 + (h2 + 1) * 512)
                        nc.tensor.matmul(
                            out=ps[:, h2 * 512 : (h2 + 1) * 512],
                            lhsT=ones_sb[:],
                            rhs=bias_sb[:, sl],
                            start=False,
                            stop=True,
                        )
                    nc.scalar.activation(
                        out=ssb[:, c * 1024 : (c + 1) * 1024], in_=ps[:], func=AF.Copy
                    )

                r8 = smpool.tile([P, K], F32, tag="r8")
                idx = smpool.tile([P, K], U32, tag="idx")
                nc.vector.max(out=r8[:], in_=ssb[:])
                nc.vector.max_index(out=idx[:], in_max=r8[:], in_values=ssb[:])

                g = gpool.tile([P, K * W], BF16)
                for k in range(K):
                    nc.gpsimd.indirect_dma_start(
                        out=g[:, k * W : (k + 1) * W],
                        out_offset=None,
                        in_=vals.ap(),
                        in_offset=bass.IndirectOffsetOnAxis(
                            ap=idx[:, k : k + 1], axis=0
                        ),
                    )
                g3 = g[:].rearrange("p (k w) -> p k w", w=W)
                d8 = smpool.tile([P, K], F32, tag="d8")
                nc.vector.tensor_sub(d8[:], r8[:], g3[:, :, H : H + 1])
                e8 = smpool.tile([P, K], F32, tag="e8")
                den = smpool.tile([P, 1], F32, tag="den")
                nc.scalar.activation(
                    out=e8[:], in_=d8[:], func=AF.Exp, scale=SCALE, accum_out=den[:]
                )
                winv = smpool.tile([P, 1], F32, tag="winv")
                nc.vector.reciprocal(out=winv[:], in_=den[:])

                osb = opool.tile([P, OUTW], F32)
                nc.vector.tensor_scalar(
                    out=osb[:, H:], in0=e8[:], scalar1=winv[:], scalar2=None,
                    op0=ALU.mult,
                )

                po = popool.tile([P, H], F32)
                for k in range(K):
                    dg = dpool.tile([P, P], BF16, tag="dg")
                    nc.scalar.activation(
                        out=dg[:], in_=ident[:], func=AF.Copy,
                        scale=e8[:, k : k + 1],
                    )
                    for h2 in range(2):
                        nc.tensor.matmul(
                            out=po[:, h2 * 512 : (h2 + 1) * 512],
                            lhsT=dg[:],
                            rhs=g[:, k * W + h2 * 512 : k * W + (h2 + 1) * 512],
                            start=(k == 0),
                            stop=(k == K - 1),
                        )
                nc.scalar.activation(
                    out=osb[:, :H], in_=po[:], func=AF.Copy, scale=winv[:]
                )
                nc.sync.dma_start(
                    out=out.ap()[i * P : (i + 1) * P, :], in_=osb[:]
                )
    nc.compile()
    return nc


def make_in_maps(query, keys, values, reliability, tpc=TPC, n_cores=N_CORES):
    query = np.asarray(query, dtype=np.float32)
    keys = np.asarray(keys, dtype=np.float32)
    values = np.asarray(values, dtype=np.float32)
    reliability = np.asarray(reliability, dtype=np.float32)
    qf = query.reshape(-1, D)
    keysT = np.ascontiguousarray(keys.T)
    bias_f = np.log(reliability.reshape(N) + EPS).astype(np.float32)
    vals16 = np.zeros((N, W), dtype=ml_dtypes.bfloat16)
    vals16[:, :H] = values.astype(ml_dtypes.bfloat16)
    vals16[:, H] = bias_f.astype(ml_dtypes.bfloat16)
    bias_row = np.ascontiguousarray(bias_f.reshape(1, N))
    rel2 = reliability.reshape(1, N)
    in_maps = []
    for c in range(n_cores):
        shard = qf[c * tpc : (c + 1) * tpc]
        in_maps.append(
            {
                "qT": np.ascontiguousarray(shard.T),
                "keysT": keysT,
                "vals": vals16,
                "rel": rel2,
                "bias_in": bias_row,
            }
        )
    return in_maps


_CACHED_NC = None


def _get_nc():
    global _CACHED_NC
    if _CACHED_NC is None:
        _CACHED_NC = build()
    return _CACHED_NC


def run(query, keys, values, reliability, trace=False, **run_kwargs):
    nc = _get_nc()
    in_maps = make_in_maps(query, keys, values, reliability)
    res = run_bass_kernel_spmd(
        nc, in_maps, core_ids=list(range(N_CORES)), trace=trace, **run_kwargs
    )
    full = np.concatenate([res.results[c]["out"] for c in range(N_CORES)], axis=0)
    output = np.ascontiguousarray(full[:, :H]).reshape(B, S, H)
    attn = np.ascontiguousarray(full[:, H:]).reshape(B, S, K)
    return (output, attn), res


def kernel(query, keys, values, reliability):
    (output, attn), _ = run(query, keys, values, reliability, trace=False)
    return output, attn
